# revision 1
# baseline (speedup 1.0000x reference)
"""Additive (Bahdanau) attention on 8 Trainium2 NeuronCores.

Math (per batch element b):
    q = query @ W_q                  [Q, H]
    k = key @ W_k                    [K, H]
    score[q_, k_] = sum_h w_v[h] * tanh(q[q_, h] + k[k_, h])
    score += -1e30 * mask            (mask fill is zeros; handled generically)
    attn = softmax(score, axis=k)
    out = attn @ value               [Q, D]

Sharding: pure data-parallel over batch B=8, one batch element per core.

Device strategy per core:
  - All matmul-consumed inputs arrive in one packed [128, PACK_N] array
    (single tile; a handful of chunked DMAs over HWDGE + SWDGE so the two
    projection input paths stream in parallel; a single dma_start fans
    out over all 16 SDMA engines at ~340 GB/s).
  - qT [H=64, Q] and kT [H=64, K] computed by PE matmuls (contraction over
    feature dim, inputs fed pre-transposed from host).
  - Key-pair packing: partitions = (2 keys x 64 heads). For key-pair p,
    DVE computes pre[p_, q] = qT2[p_, q] + kT_pairs[p_, p] (per-partition
    scalar add, 2x fp32 mode; during the ramp every other add runs on the
    otherwise-idle GPSIMD so supply keeps pace), ACT computes tanh on up
    to [128, 16*512] tiles (group sizes tapered small at the start for
    fast ramp-up and at the end to shrink the post-tanh tail). ACT is the bottleneck engine:
    1 elem/lane/cycle @ 1.2 GHz over the B*Q*K*H = 134M-element tanh
    volume (~109 us/core streaming floor; ~120 us busy incl. overheads).
  - PE reduces over h with the sliding w_sel strip (w_v pair at columns
    126/127 of a [128, 254] zero strip; lhsT slice [126-2j : 254-2j]
    places the pair at output rows 2j, 2j+1), accumulating 64 float32r
    matmuls (1 cycle/row; plain fp32 would be 4) per scores PSUM bank in
    [k-part, q-free] layout.
  - scores [k, q] layout makes softmax denominators a ones-column in the
    AV matmul: value_ext = [value | 1 | 0]. No transposes and no
    max-subtraction needed (|score| <= sum|w_v| ~ 7, exp is safe; masked
    entries are -1e30 -> exp == 0 exactly).
  - AV is incremental: as each k-block's exp tile appears, it is folded
    into four per-q-block PSUM accumulators; col 256 = softmax sums.
    Normalize with DVE reciprocal + per-partition scale on ACT/DVE.
"""

import numpy as np
from contextlib import ExitStack

import concourse.bass as bass
import concourse.tile as tile
from concourse import bacc
from concourse import mybir
from concourse import bass_utils

B, Q, K, H, D = 8, 512, 512, 64, 256
N_CORES = 8

# packed layout (columns of a [128, PACK_N] f32 array)
# W blocks are duplicated to 128 columns (fp32r matmuls require full
# col_grp 0xf AND the duplicate makes the projection emit qT stacked twice
# on partitions directly). The moving operand innermost count must be even
# (fp32r streams two fp32 per port-cycle): VAL_W = D + 2, zero pad col.
PACK_PROJ = 128 + 512   # per d-block: [W duplicated to 128 cols | xT (512)]
VAL_W = D + 2           # value | ones | zero-pad  (even innermost count)
OFF_Q = 0               # 2 d-blocks of query side
OFF_K = 2 * PACK_PROJ   # 2 d-blocks of key side
# w_sel strip: [128, 254] zeros except col 126 = [w_v; 0], col 127 = [0; w_v].
# lhsT slice [126-2j : 254-2j] places the w_v pair at output rows (2j, 2j+1)
# of a full 128-row matmul; all other output rows get zeros (accumulate-safe).
OFF_WSEL = OFF_K + 2 * PACK_PROJ          # 2304
WSEL_N = 254
OFF_VAL = OFF_WSEL + WSEL_N
PACK_N = OFF_VAL + 4 * VAL_W

F32 = mybir.dt.float32
F32R = mybir.dt.float32r
AF = mybir.ActivationFunctionType


def _emit(ctx, tc, nc, ins, out_d, reps=1):
    const = ctx.enter_context(tc.tile_pool(name="const", bufs=1))
    pre_pool = ctx.enter_context(tc.tile_pool(name="pre", bufs=2))
    feat_pool = ctx.enter_context(tc.tile_pool(name="feat", bufs=2))
    sc_pool = ctx.enter_context(tc.tile_pool(name="scsb", bufs=2))
    out_pool = ctx.enter_context(tc.tile_pool(name="outp", bufs=4))
    ps_proj = ctx.enter_context(
        tc.tile_pool(name="ps_proj", bufs=2, space=bass.MemorySpace.PSUM))
    ps_sc = ctx.enter_context(
        tc.tile_pool(name="ps_sc", bufs=2, space=bass.MemorySpace.PSUM))
    ps_av = ctx.enter_context(
        tc.tile_pool(name="ps_av", bufs=4, space=bass.MemorySpace.PSUM))

    # ---- persistent tiles ----
    # One SBUF tile per DMA chunk: exact per-chunk dependencies, so the
    # first projection matmul starts as soon as its own chunk lands. Each
    # side's first chunk goes via HWDGE (fast prep) and second via SWDGE,
    # so the two descriptor-generation engines prep in parallel.
    pq = [const.tile([128, PACK_PROJ], F32R, name=f"pq{db}") for db in range(2)]
    pk = [const.tile([128, PACK_PROJ], F32R, name=f"pk{db}") for db in range(2)]
    prest = const.tile([128, PACK_N - OFF_WSEL], F32R)
    for db, eng in ((0, nc.sync), (1, nc.gpsimd)):
        eng.dma_start(
            pq[db][:], ins["packed"].ap()
            [:, OFF_Q + db * PACK_PROJ:OFF_Q + (db + 1) * PACK_PROJ]
            .bitcast(F32R))
    for db, eng in ((0, nc.sync), (1, nc.gpsimd)):
        eng.dma_start(
            pk[db][:], ins["packed"].ap()
            [:, OFF_K + db * PACK_PROJ:OFF_K + (db + 1) * PACK_PROJ]
            .bitcast(F32R))
    nc.gpsimd.dma_start(prest[:],
                        ins["packed"].ap()[:, OFF_WSEL:PACK_N].bitcast(F32R))

    maskn_sb = const.tile([128, 4 * Q], F32)

    warm = const.tile([128, 2], F32)
    nc.vector.memset(warm[:], 0.0)
    nc.scalar.activation(warm[:], warm[:], AF.Tanh)  # pull ACT_TABLE_LOAD to t~0

    qT2 = const.tile([128, Q], F32)       # qT stacked twice on partitions
    kT_pairs = const.tile([128, K // 2], F32)  # [kT[:,2i]; kT[:,2i+1]]
    exp_sb = const.tile([128, 4 * Q], F32R)    # exp(scores), [k-part, q]

    for rep in range(reps):
        # ---- projections: qT = W_q.T @ queryT, kT = W_k.T @ keyT ----
        qps = ps_proj.tile([128, Q], F32, tag="proj")
        for db in range(2):
            nc.tensor.matmul(qps[:], pq[db][:, 0:128],
                             pq[db][:, 128:PACK_PROJ],
                             start=(db == 0), stop=(db == 1))
        # W block is duplicated into lhsT cols 64:128 host-side, so qps
        # rows 64:128 already hold a second copy of qT -> one full copy
        nc.scalar.copy(qT2[:, :], qps[:, :])

        kps = ps_proj.tile([128, K], F32, tag="proj")
        for db in range(2):
            nc.tensor.matmul(kps[:], pk[db][:, 0:128],
                             pk[db][:, 128:PACK_PROJ],
                             start=(db == 0), stop=(db == 1))
        # de-interleave key pairs: kT_pairs[0:64, i] = kT[:, 2i] (from kps
        # top half), [64:128, i] = kT[:, 2i+1] (from the duplicated bottom)
        kps3t = kps[0:64, :].rearrange("h (i two) -> h two i", two=2)
        kps3b = kps[64:128, :].rearrange("h (i two) -> h two i", two=2)
        nc.vector.tensor_copy(kT_pairs[0:64, 0:32], kps3t[:, 0:1, 0:32])
        nc.vector.tensor_copy(kT_pairs[64:128, 0:32], kps3b[:, 1:2, 0:32])

        if rep == 0:
            nc.sync.dma_start(
                maskn_sb[:].rearrange("p (kb q) -> p kb q", kb=4),
                ins["maskn"].ap().rearrange("(kb p) q -> p kb q", kb=4))

        # ---- main loop: tanh volume + w_v reduction ----
        # Tapered group sizes: small groups at the start (ACT begins sooner
        # after fewer DVE adds) and at the end (the final scores matmul
        # chunk after the last tanh is small, so the last exp starts early).
        sizes = [2, 2, 4, 8, 16, 16, 16] + [16] * 10 + [16, 8, 4, 2, 2]
        assert sum(sizes) == K // 2
        scores_ps = [None] * 4
        p0 = 0
        for g, npair in enumerate(sizes):
            pre = pre_pool.tile([128, npair * Q], F32, tag="pre",
                                name=f"pre_{rep}_{g}")
            for t in range(npair):
                p = p0 + t
                # during the ramp, route a slice of the adds to the idle
                # GPSIMD engine so DVE supply keeps up with ACT's growing
                # group sizes
                eng = nc.gpsimd if (2 <= g <= 8 and t % 2 == 1) else nc.vector
                eng.tensor_scalar_add(pre[:, Q * t:Q * (t + 1)], qT2[:],
                                      kT_pairs[:, p:p + 1])
            feat = feat_pool.tile([128, npair * Q], F32R, tag="feat",
                                  name=f"feat_{rep}_{g}")
            nc.scalar.activation(feat[:], pre[:], AF.Tanh)
            for t in range(npair):
                p = p0 + t
                kb = p // 64
                if p % 64 == 0:
                    scores_ps[kb] = ps_sc.tile([128, Q], F32, tag="scores",
                                               name=f"scores_{rep}_{g}")
                jj = p % 64
                nc.tensor.matmul(
                    scores_ps[kb][:],
                    prest[:, 126 - 2 * jj:254 - 2 * jj],
                    feat[:, Q * t:Q * (t + 1)],
                    start=(jj == 0), stop=(jj == 63))
            p0 += npair
            # stream in the remaining kT_pairs columns between early groups
            # so the bulk copies don't block the ramp-up adds on DVE
            if g == 2:
                nc.scalar.copy(kT_pairs[0:64, 32:128], kps3t[:, 0:1, 32:128])
                nc.scalar.copy(kT_pairs[64:128, 32:128], kps3b[:, 1:2, 32:128])
            if g == 5:
                nc.vector.tensor_copy(kT_pairs[0:64, 128:256],
                                      kps3t[:, 0:1, 128:256])
                nc.vector.tensor_copy(kT_pairs[64:128, 128:256],
                                      kps3b[:, 1:2, 128:256])
            if p0 % 64 == 0:  # one k-block of scores is complete
                kb = p0 // 64 - 1
                halves = ((0, Q),) if kb < 3 else ((0, Q // 2), (Q // 2, Q))
                sc = sc_pool.tile([128, Q], F32)
                for (h0, h1) in halves:
                    nc.vector.tensor_add(
                        sc[:, h0:h1], scores_ps[kb][:, h0:h1],
                        maskn_sb[:, kb * Q + h0:kb * Q + h1])
                    nc.scalar.activation(
                        exp_sb[:, kb * Q + h0:kb * Q + h1], sc[:, h0:h1],
                        AF.Exp)
                    # incremental AV: fold this k-block into the q-block
                    # accumulators as soon as their exp columns exist
                    if kb == 0 and h0 == 0:
                        avps = [ps_av.tile([128, VAL_W], F32, tag="avp",
                                           name=f"avp_{rep}_{qb}")
                                for qb in range(4)]
                    for qb in range(h0 // 128, h1 // 128):
                        nc.tensor.matmul(
                            avps[qb][:],
                            exp_sb[:, kb * Q + 128 * qb:
                                   kb * Q + 128 * (qb + 1)],
                            prest[:, WSEL_N + kb * VAL_W:
                                   WSEL_N + (kb + 1) * VAL_W],
                            start=(kb == 0), stop=(kb == 3))

        # ---- normalize + store ----
        for qb in range(4):
            avp = avps[qb]
            r = out_pool.tile([128, 1], F32, tag="recip", name=f"r_{rep}_{qb}")
            nc.vector.reciprocal(r[:], avp[:, D:D + 1])
            osb = out_pool.tile([128, D], F32, tag="osb", name=f"osb_{rep}_{qb}")
            if qb % 2 == 0:
                nc.scalar.activation(osb[:], avp[:, 0:D], AF.Identity,
                                     bias=0.0, scale=r[:, 0:1])
            else:
                nc.vector.tensor_scalar_mul(osb[:], avp[:, 0:D], r[:, 0:1])
            out_dma = nc.gpsimd.dma_start if qb % 2 == 0 else nc.sync.dma_start
            out_dma(out_d.ap()[128 * qb:128 * (qb + 1), :], osb[:])


def _build_bass(reps=1):
    nc = bacc.Bacc("TRN2", target_bir_lowering=False, debug=False,
                   enable_asserts=False, num_devices=N_CORES)
    ins = {
        "packed": nc.dram_tensor("packed", [128, PACK_N], F32,
                                 kind="ExternalInput"),
        "maskn": nc.dram_tensor("maskn", [K, Q], F32, kind="ExternalInput"),
    }
    out_d = nc.dram_tensor("out", [Q, D], F32, kind="ExternalOutput")
    with tile.TileContext(nc) as tc, ExitStack() as ctx:
        _emit(ctx, tc, nc, ins, out_d, reps=reps)
    nc.compile()
    return nc


_NC_CACHE = None


def _get_nc():
    global _NC_CACHE
    if _NC_CACHE is None:
        _NC_CACHE = _build_bass()
    return _NC_CACHE


def make_in_maps(key, query, value, mask, W_k, W_q, w_v):
    key = np.asarray(key, dtype=np.float32)
    query = np.asarray(query, dtype=np.float32)
    value = np.asarray(value, dtype=np.float32)
    mask = np.asarray(mask)
    W_k = np.asarray(W_k, dtype=np.float32)
    W_q = np.asarray(W_q, dtype=np.float32)
    w_v = np.asarray(w_v, dtype=np.float32)

    w_sel = np.zeros((128, WSEL_N), dtype=np.float32)
    w_sel[0:64, 126] = w_v
    w_sel[64:128, 127] = w_v

    in_maps = []
    for b in range(B):
        queryT = query[b].T  # [D, Q]
        keyT = key[b].T      # [D, K]
        packed = np.zeros((128, PACK_N), dtype=np.float32)
        for db in range(2):
            base = OFF_Q + db * PACK_PROJ
            packed[:, base:base + H] = W_q[db * 128:(db + 1) * 128, :]
            packed[:, base + H:base + 128] = W_q[db * 128:(db + 1) * 128, :]
            packed[:, base + 128:base + PACK_PROJ] = queryT[db * 128:(db + 1) * 128, :]
            base = OFF_K + db * PACK_PROJ
            packed[:, base:base + H] = W_k[db * 128:(db + 1) * 128, :]
            packed[:, base + H:base + 128] = W_k[db * 128:(db + 1) * 128, :]
            packed[:, base + 128:base + PACK_PROJ] = keyT[db * 128:(db + 1) * 128, :]
        packed[:, OFF_WSEL:OFF_WSEL + WSEL_N] = w_sel
        for kb in range(4):
            base = OFF_VAL + kb * VAL_W
            packed[:, base:base + D] = value[b, kb * 128:(kb + 1) * 128, :]
            packed[:, base + D] = 1.0
        in_maps.append({
            "packed": packed,
            "maskn": np.ascontiguousarray(
                mask[b].T.astype(np.float32) * np.float32(-1e30)),
        })
    return in_maps


def kernel(key, query, value, mask, W_k, W_q, w_v):
    nc = _get_nc()
    in_maps = make_in_maps(key, query, value, mask, W_k, W_q, w_v)
    res = bass_utils.run_bass_kernel_spmd(nc, in_maps, core_ids=list(range(N_CORES)))
    return np.stack([res.results[c]["out"] for c in range(N_CORES)], axis=0)

